# revision 22
# baseline (speedup 1.0000x reference)
# A_n lattice quantizer kernel for Trainium2 (8 NeuronCores, data-parallel).
#
# reference:
#   xp = x @ transform            [B, 257]
#   f = round(xp); Delta = sum(f) per row
#   delta = xp - f; correct the Delta smallest (Delta>0) / |Delta| largest
#   (Delta<0) residuals by -/+1 so each row of f sums to zero
#   out = f @ transform.T         [B, 256]
#
# Key identity used here: transform @ transform.T = I (orthonormal rows), so
#   out = x + (e - z1) @ transform.T,   e = round(xp) - xp,  z1 = s*1{w>u}
# The backward operand m = e - z1 = s*(w - z01) is small-magnitude, so the
# whole backward path runs in single bf16 (no hi/lo split) and f is never
# materialized. x is added back on the host.
#
# Per 128-row tile (rows on partitions, 257 on free dim):
#   fwd matmul bf16 hi/lo (x pre-transposed + packed on host)  -> xp PSUM
#   g  = xp + C (ACT, C = 1.5*2^23 magic round)
#   e  = (g - C) - xp, accum -> se ~= Delta            (DVE STT)
#   s  = Sign(se) in {-1,+1}  (ACT); w = s*e (ACT, per-partition scale)
#   top-16 of w via max8 + match_replace + max8        (DVE, f32)
#   u  = w-value at rank |Dd| via one-hot dot with iota (|Dd| clamped to 15)
#   mneg = 1{w>u} - w                                  (GpSimd STT, bf16)
#   PE transposes of mneg; fT = -mnegT (ACT copy scale=-1) = (w - z01)^T
#   out_ps = fT @ trT (bf16); o16 = s * out_ps (ACT copy scale=s, bf16)
# Host: out = x + o16
import numpy as np

try:  # make concourse importable in bare environments
    import concourse  # noqa: F401
except ImportError:
    import sys

    for _p in ("/opt/trn_rl_repo", "/root/.axon_site/_ro/trn_rl_repo"):
        if _p not in sys.path:
            sys.path.append(_p)

DIM = 256
N1 = 257
P = 128
BATCH = 262144
NCORES = 8
ROWS_PER_CORE = BATCH // NCORES

C_MAGIC = 12582912.0  # 1.5 * 2**23
NEG_BIG = -1e30
# selection depth: 2 rounds of max8 -> top-16; rows with |Delta| > 15 (~6e-4
# of rows) get only their 15 most extreme coords corrected (tiny rel-err).
KSEL = 16
DMAX_CLAMP = 15.4  # clamp se before rounding; rint gives |Dd| <= 15

_CACHE = {}


def _build(rows, mm_dtype_name="float32"):
    from contextlib import ExitStack

    import concourse.bass as bass
    import concourse.tile as tile
    from concourse import bacc, mybir
    from concourse.masks import make_identity

    f32 = mybir.dt.float32
    bf16 = mybir.dt.bfloat16
    op = mybir.AluOpType
    act = mybir.ActivationFunctionType

    nc = bacc.Bacc(
        "TRN2", target_bir_lowering=False, debug=False, num_devices=NCORES
    )
    xthl_d = nc.dram_tensor(
        "xthl", [rows, 2 * DIM], bf16, kind="ExternalInput"
    ).ap()
    trh_d = nc.dram_tensor("trh", [DIM, N1], bf16, kind="ExternalInput").ap()
    trl_d = nc.dram_tensor("trl", [DIM, N1], bf16, kind="ExternalInput").ap()
    trTb_d = nc.dram_tensor("trTb", [N1, DIM], bf16, kind="ExternalInput").ap()
    iota_d = nc.dram_tensor("iota", [P, KSEL], f32, kind="ExternalInput").ap()
    out_d = nc.dram_tensor("out", [rows, DIM], bf16, kind="ExternalOutput").ap()

    ntiles = rows // P

    with tile.TileContext(nc) as tc, ExitStack() as ctx:
        const = ctx.enter_context(tc.tile_pool(name="const", bufs=1))
        xin = ctx.enter_context(tc.tile_pool(name="xin", bufs=8))
        sb = ctx.enter_context(tc.tile_pool(name="sb", bufs=8))
        smalls = ctx.enter_context(tc.tile_pool(name="smalls", bufs=8))
        outp = ctx.enter_context(tc.tile_pool(name="outp", bufs=6))
        ps_xp = ctx.enter_context(tc.tile_pool(name="ps_xp", bufs=3, space="PSUM"))
        ps_ft = ctx.enter_context(tc.tile_pool(name="ps_ft", bufs=2, space="PSUM"))
        ps_out = ctx.enter_context(tc.tile_pool(name="ps_out", bufs=3, space="PSUM"))

        # --- constants ---
        ident = const.tile([P, P], f32)
        make_identity(nc, ident[:])
        identb = const.tile([P, P], bf16)
        nc.vector.tensor_copy(identb[:], ident[:])
        trh_sb = const.tile([P, 2 * N1], bf16)
        nc.sync.dma_start(out=trh_sb[:, 0:N1], in_=trh_d[0:P, :])
        nc.sync.dma_start(out=trh_sb[:, N1 : 2 * N1], in_=trh_d[P:DIM, :])
        trl_sb = const.tile([P, 2 * N1], bf16)
        nc.sync.dma_start(out=trl_sb[:, 0:N1], in_=trl_d[0:P, :])
        nc.sync.dma_start(out=trl_sb[:, N1 : 2 * N1], in_=trl_d[P:DIM, :])
        trT_sb = const.tile([P, 2 * DIM], bf16)
        nc.sync.dma_start(out=trT_sb[:, 0:DIM], in_=trTb_d[0:P, :])
        nc.sync.dma_start(out=trT_sb[:, DIM : 2 * DIM], in_=trTb_d[P:DIM, :])
        trT3_sb = const.tile([1, DIM], bf16)
        nc.sync.dma_start(out=trT3_sb[:], in_=trTb_d[DIM:N1, :])
        iota_sb = const.tile([P, KSEL], f32)
        nc.sync.dma_start(out=iota_sb[:], in_=iota_d[:])
        cbias = const.tile([P, 1], f32)
        nc.gpsimd.memset(cbias[:], C_MAGIC)

        def phase1(i):
            r0 = i * P
            # load pre-transposed bf16 hi/lo lhsT tile (host-packed, 1 DMA)
            xt = xin.tile([P, 4 * P], bf16, tag="xt")
            nc.sync.dma_start(out=xt[:], in_=xthl_d[r0 : r0 + P, :])
            xTh = xt[:, 0 : 2 * P]
            xTl = xt[:, 2 * P : 4 * P]

            # forward matmul: xp = xh@th + xh@tl + xl@th  [128, 257]
            xp = ps_xp.tile([P, N1], f32, tag="xp")
            nc.tensor.matmul(
                xp[:], xTh[:, 0:P], trh_sb[:, 0:N1], start=True, stop=False
            )
            nc.tensor.matmul(
                xp[:], xTh[:, 0:P], trl_sb[:, 0:N1], start=False, stop=False
            )
            nc.tensor.matmul(
                xp[:], xTl[:, 0:P], trh_sb[:, 0:N1], start=False, stop=False
            )
            nc.tensor.matmul(
                xp[:], xTh[:, P : 2 * P], trh_sb[:, N1 : 2 * N1],
                start=False, stop=False,
            )
            nc.tensor.matmul(
                xp[:], xTh[:, P : 2 * P], trl_sb[:, N1 : 2 * N1],
                start=False, stop=False,
            )
            nc.tensor.matmul(
                xp[:], xTl[:, P : 2 * P], trh_sb[:, N1 : 2 * N1],
                start=False, stop=True,
            )

            # g = xp + C  (ACT, PSUM->SBUF)
            g = sb.tile([P, N1], f32, tag="g")
            nc.scalar.activation(g[:], xp[:], act.Identity, bias=cbias[:])
            # e = (g - C) - xp ; accum -> se ~= Delta
            e = sb.tile([P, N1], f32, tag="e")
            se = smalls.tile([P, 1], f32, tag="se")
            nc.vector.scalar_tensor_tensor(
                out=e[:], in0=g[:], scalar=C_MAGIC, in1=xp[:],
                op0=op.subtract, op1=op.subtract, accum_out=se[:],
            )
            # s2 = 1{se>=0} - 0.5 in {-0.5, +0.5} = s/2  (gpsimd, const
            # scalars only — Pool rejects per-partition-scalar TS). The
            # factor 2 is folded into the fT copy scale (-4.0) below.
            s2 = smalls.tile([P, 1], f32, tag="s2")
            nc.gpsimd.tensor_scalar(
                out=s2[:], in0=se[:], scalar1=0.0, scalar2=0.5,
                op0=op.is_ge, op1=op.subtract,
            )
            # sec = clamp(se, +-15.4); Dd = round(sec); aDd = |Dd|
            # sec/Dd are [P,1] const-scalar ops consumed a phase later —
            # run them on gpsimd to keep DVE (the pacing engine) clear
            sec = smalls.tile([P, 1], f32, tag="sec")
            nc.gpsimd.tensor_scalar(
                out=sec[:], in0=se[:], scalar1=-DMAX_CLAMP, scalar2=DMAX_CLAMP,
                op0=op.max, op1=op.min,
            )
            Dd = smalls.tile([P, 1], f32, tag="Dd")
            nc.gpsimd.tensor_scalar(
                out=Dd[:], in0=sec[:], scalar1=C_MAGIC, scalar2=C_MAGIC,
                op0=op.add, op1=op.subtract,
            )
            # w = s2 * e = 0.5*s*e  (ACT: per-partition scale; halved scale
            # preserves selection order, compensated at fT/o16)
            w = sb.tile([P, N1], f32, tag="w")
            nc.scalar.mul(w[:], e[:], s2[:])
            return dict(w=w, s2=s2, Dd=Dd, i=i)

        def phase2(st):
            w, s2, Dd = st["w"], st["s2"], st["Dd"]
            r0 = st["i"] * P
            # top-16 of w, sorted desc, via 2x max8 + 1x match_replace
            v24 = smalls.tile([P, KSEL], f32, tag="v24")
            wm1 = sb.tile([P, N1], f32, tag="wm1")
            nc.vector.max(v24[:, 0:8], w[:])
            nc.vector.match_replace(wm1[:], v24[:, 0:8], w[:], NEG_BIG)
            nc.vector.max(v24[:, 8:16], wm1[:])
            # u = v24[|Dd|] via one-hot: iota holds 2k, so 2k*s2 = k*s == Dd
            ind = smalls.tile([P, KSEL], f32, tag="ind")
            nc.vector.tensor_scalar(
                out=ind[:], in0=iota_sb[:], scalar1=s2[:], scalar2=Dd[:],
                op0=op.mult, op1=op.is_equal,
            )
            scr = smalls.tile([P, KSEL], f32, tag="scr")
            u = smalls.tile([P, 1], f32, tag="u")
            nc.vector.scalar_tensor_tensor(
                out=scr[:], in0=ind[:], scalar=1.0, in1=v24[:],
                op0=op.mult, op1=op.mult, accum_out=u[:],
            )
            # z01 = 1{w>u} (DVE TS, single-src 2x mode); mneg = z01 - w on
            # gpsimd (Pool rejects per-partition-scalar TS, plain TT is fine)
            # z01 = 0.5*1{w>u} so mneg = z01 - w = 0.5*(1{..} - s*e)
            z01 = sb.tile([P, N1], f32, tag="z01")
            nc.vector.tensor_scalar(
                out=z01[:], in0=w[:], scalar1=u[:], scalar2=0.5,
                op0=op.is_gt, op1=op.mult,
            )
            mneg = sb.tile([P, N1], bf16, tag="mneg")
            nc.gpsimd.tensor_sub(out=mneg[:], in0=z01[:], in1=w[:])

            # transpose mneg (257 cols -> 3 chunks into one PSUM bank)
            ft_ps = ps_ft.tile([P, 3 * P], bf16, tag="ft_ps")
            nc.tensor.matmul(
                ft_ps[:, 0:P], mneg[:, 0:P], identb[:],
                is_transpose=True, start=True, stop=False,
            )
            nc.tensor.matmul(
                ft_ps[:, P : 2 * P], mneg[:, P : 2 * P], identb[:],
                is_transpose=True, start=False, stop=False,
            )
            nc.tensor.matmul(
                ft_ps[0:1, 2 * P : 3 * P], mneg[:, 2 * P : N1],
                identb[:], is_transpose=True, start=False, stop=True,
            )
            # fT = -4*mnegT = 2*(s*e - 1{w>u})^T ; the 2 compensates s2=s/2
            fT = sb.tile([P, 3 * P], bf16, tag="fT")
            nc.scalar.activation(fT[:], ft_ps[:], act.Copy, scale=-4.0)

            # backward matmul: out_ps = (w - z01) @ trT  [128, 256]
            o_ps = ps_out.tile([P, DIM], f32, tag="o_ps")
            nc.tensor.matmul(
                o_ps[:], fT[:, 0:P], trT_sb[:, 0:DIM], start=True, stop=False
            )
            nc.tensor.matmul(
                o_ps[:], fT[:, P : 2 * P], trT_sb[:, DIM : 2 * DIM],
                start=False, stop=False,
            )
            nc.tensor.matmul(
                o_ps[:], fT[0:1, 2 * P : 3 * P], trT3_sb[:],
                start=False, stop=True,
            )
            # o16 = s2 * out_ps = s2*2*(s*e - z01)@trT = (e - z1)@trT  (bf16)
            o_sb = outp.tile([P, DIM], bf16, tag="o_sb")
            nc.scalar.activation(o_sb[:], o_ps[:], act.Copy, scale=s2[:])
            nc.sync.dma_start(out=out_d[r0 : r0 + P, :], in_=o_sb[:])

        prev = None
        for i in range(ntiles + 1):
            cur = phase1(i) if i < ntiles else None
            if prev is not None:
                phase2(prev)
            prev = cur

    nc.finalize()
    return nc


def _get_nc(rows, mm_dtype_name="float32"):
    key = (rows, mm_dtype_name)
    if key not in _CACHE:
        _CACHE[key] = _build(rows, mm_dtype_name)
    return _CACHE[key]


def _pack_xT(x_shard):
    rows = x_shard.shape[0]
    nt = rows // P
    return np.ascontiguousarray(
        x_shard.reshape(nt, P, 2, P).transpose(0, 3, 2, 1).reshape(rows, DIM)
    )


def _split_pack_x(x_shard):
    import ml_dtypes

    xh = x_shard.astype(ml_dtypes.bfloat16)
    xl = (x_shard - xh.astype(np.float32)).astype(ml_dtypes.bfloat16)
    return np.ascontiguousarray(
        np.concatenate([_pack_xT(xh), _pack_xT(xl)], axis=1)
    )


def _split_tr(transform):
    import ml_dtypes

    th = transform.astype(ml_dtypes.bfloat16)
    tl = (transform - th.astype(np.float32)).astype(ml_dtypes.bfloat16)
    return th, tl


def _trT_bf16(transform):
    import ml_dtypes

    return np.ascontiguousarray(transform.T.astype(ml_dtypes.bfloat16))


def _make_consts():
    # iota holds 2k so that 2k * s2 (= k*s) matches the signed integer Dd
    iota = np.broadcast_to(
        (2.0 * np.arange(KSEL, dtype=np.float32))[None, :], (P, KSEL)
    ).copy()
    return iota


def kernel(x, transform, _rows=None, _mm_dtype="float32", _trace=False):
    from concourse.bass_utils import run_bass_kernel_spmd

    x = np.ascontiguousarray(np.asarray(x, dtype=np.float32))
    transform = np.ascontiguousarray(np.asarray(transform, dtype=np.float32))
    batch = x.shape[0]
    rows = _rows if _rows is not None else batch // NCORES
    assert rows % P == 0 and batch == rows * NCORES

    nc = _get_nc(rows, _mm_dtype)
    trh, trl = _split_tr(transform)
    trTb = _trT_bf16(transform)
    iota = _make_consts()
    in_maps = []
    for c in range(NCORES):
        in_maps.append(
            {
                "xthl": _split_pack_x(x[c * rows : (c + 1) * rows]),
                "trh": trh,
                "trl": trl,
                "trTb": trTb,
                "iota": iota,
            }
        )
    res = run_bass_kernel_spmd(
        nc, in_maps, core_ids=list(range(NCORES)), trace=_trace
    )
    out = np.concatenate(
        [res.results[c]["out"].astype(np.float32) for c in range(NCORES)],
        axis=0,
    )
    out += x  # out = x + (e - z1) @ trT
    if _trace:
        return out, res
    return out
